# revision 33
# baseline (speedup 1.0000x reference)
"""Trainium2 Bass kernel for nn_AittalaGCN1dBlock (3-layer GCN block stack).

Self-contained: kernel(**inputs) takes FULL inputs, returns FULL output.

Strategy
--------
- GCN message passing as a dense [2048 x 2048] adjacency matmul (gcn_norm
  folded in, built on host; PE 128x128 MACs beat any sparse gather path).
- Data-parallel over graphs: core k handles b = k//2, n in [8*(k%2), +8).
  The "amax" half of every concat is a single shared tile per core.
- Cross-node max over n: AllReduce(max) within core pairs {2b, 2b+1}.
- All tensor data fp16 (full-rate matmul), fp32 PSUM accumulation.
- Boundary pipeline (collective latency ~10-13us/AR, serial per-core
  channel; NB the engine-queue/trigger order is decided by the Tile
  scheduler's readiness simulation, NOT by emission order — A/B runs
  showed fake-dependency or consolidation games around the channel
  reliably backfire):
    * ch0-2 amax pair-ARs fire per-chunk and overlap compute; at block
      end: st1 8-way AR (aA-half stats), ch3 pair-AR, st2 8-way AR
      (amax-half stats, needs ch3).
    * The aA-half normalize depends only on st1, so the next block's
      per-graph relus + W-matmuls (psw) start while the ch3/st2 chain
      resolves.
    * The amax-half contribution to the next block's GCN is contracted
      SEPARATELY: shared_agg[c,dst] = sum_st (relu_amax^T W).T A — one
      "virtual graph" A-pass (+13.6us PE/block). Per-graph evacs then do
      aA = psum + shared_agg on DVE, decoupling the per-graph W-matmuls
      and the whole per-graph A-contraction from the amax chain; the
      A-phase holds 6 ch0 psums open until shared_agg[ch0] lands
      (pools: psum_a=6 / psum_w=2).
- Tail: bulk relu + output DMA keyed off st1; amax relu + out2 off st2.
"""

import os
import numpy as np

B, N, L = 4, 16, 2048
C0 = 64
NCORES = 8
EPS = 1e-5
PAIRS = [[0, 1], [2, 3], [4, 5], [6, 7]]
ALLCORES = [list(range(NCORES))]

_CACHE = {}


def _build_A_T(edge_index):
    """A_T[src, dst]: out[:, dst] = sum_src hW[src, :] * A_T[src, dst].

    PyG gcn_norm with improved=True: self loops weight 2, symmetric norm.
    """
    src = np.asarray(edge_index[0], np.int64)
    dst = np.asarray(edge_index[1], np.int64)
    deg = np.zeros(L, np.float32)
    np.add.at(deg, dst, np.float32(1.0))
    deg += 2.0
    dinv = (1.0 / np.sqrt(deg)).astype(np.float32)
    A_T = np.zeros((L, L), np.float32)
    np.add.at(A_T, (src, dst), dinv[src] * dinv[dst])
    idx = np.arange(L)
    A_T[idx, idx] += 2.0 * dinv * dinv
    return A_T


def _build_nc():
    from contextlib import ExitStack
    from concourse import bass, mybir, tile, bacc

    dt = mybir.dt
    F16, F32 = dt.float16, dt.float32
    AF = mybir.ActivationFunctionType
    ALU = mybir.AluOpType

    nc = bacc.Bacc("TRN2", target_bir_lowering=False, debug=False,
                   num_devices=NCORES)

    x_in = nc.dram_tensor("x_pack", [128, 4, 16, 128], F16,
                          kind="ExternalInput").ap()
    a_in = nc.dram_tensor("a_t", [4, 128, 16, 512], F16,
                          kind="ExternalInput").ap()
    w1_in = nc.dram_tensor("w1p", [128, 2, 128], F16, kind="ExternalInput").ap()
    w2_in = nc.dram_tensor("w2c", [128, 2, 128], F16, kind="ExternalInput").ap()
    w3_in = nc.dram_tensor("w3c", [128, 2, 128], F16, kind="ExternalInput").ap()
    par_in = nc.dram_tensor("par", [3, 128, 6], F32, kind="ExternalInput").ap()
    out_dram = nc.dram_tensor("out", [8, 128, L], F16, kind="ExternalOutput").ap()
    out2_dram = nc.dram_tensor("out2", [128, L], F16, kind="ExternalOutput").ap()

    # collective bounce buffers (DRAM only; SBUF collectives are banned).
    # Shared outputs are rejected for 2-rank groups; Local is fine there.
    amax_in_d = [[nc.dram_tensor(f"amax_in{k}_{ch}", [128, 512], F16)
                  for ch in range(4)] for k in range(3)]
    amax_out_d = [[nc.dram_tensor(f"amax_out{k}_{ch}", [128, 512], F16)
                   for ch in range(4)] for k in range(3)]
    st1_in_d = [nc.dram_tensor(f"st1_in{k}", [128, 2], F32) for k in range(3)]
    st1_out_d = [nc.dram_tensor(f"st1_out{k}", [128, 2], F32,
                                addr_space="Shared") for k in range(3)]
    st2_in_d = [nc.dram_tensor(f"st2_in{k}", [128, 2], F32) for k in range(3)]
    st2_out_d = [nc.dram_tensor(f"st2_out{k}", [128, 2], F32,
                                addr_space="Shared") for k in range(3)]
    # NB: do NOT fire a priming collective at t=0 — the initial ~38us channel
    # BARRIER start time is runtime-fixed; a dummy AR only queues ahead of
    # block-1's real ARs on the serial channel (A/B-measured regression).

    with tile.TileContext(nc) as tc, ExitStack() as ctx:
        const = ctx.enter_context(tc.tile_pool(name="const", bufs=1))
        psum_a = ctx.enter_context(tc.tile_pool(name="psum_a", bufs=6,
                                                space="PSUM"))
        psum_w = ctx.enter_context(tc.tile_pool(name="psum_w", bufs=2,
                                                space="PSUM"))
        hbufs = ctx.enter_context(tc.tile_pool(name="hbufs", bufs=1))
        work = ctx.enter_context(tc.tile_pool(name="work", bufs=2))
        blk1 = ctx.enter_context(tc.tile_pool(name="blk1", bufs=1))

        # ---- resident constants, loaded in consumption order ----
        # two HWDGE rings in parallel: x on the sync ring, A on the scalar
        # ring. A ch0 lands in 4 st-slices so the first matmul group can
        # start ~3us in.
        A_sb = const.tile([128, 4, 16, 512], F16)
        x_sb = blk1.tile([128, 4, 16, 128], F16)
        nc.sync.dma_start(x_sb[:, 0], x_in[:, 0])
        for q in range(4):
            nc.scalar.dma_start(A_sb[:, 0, 4 * q:4 * q + 4],
                                a_in[0, :, 4 * q:4 * q + 4])
        nc.sync.dma_start(x_sb[:, 1:4], x_in[:, 1:4])
        for ch in range(1, 4):
            nc.scalar.dma_start(A_sb[:, ch], a_in[ch])
        W1_sb = const.tile([128, 2, 128], F16)
        nc.sync.dma_start(W1_sb, w1_in)
        W2_sb = const.tile([128, 2, 128], F16)
        nc.sync.dma_start(W2_sb, w2_in)
        W3_sb = const.tile([128, 2, 128], F16)
        nc.sync.dma_start(W3_sb, w3_in)
        par_sb = const.tile([128, 3, 6], F32)
        nc.sync.dma_start(par_sb, par_in.rearrange("k p f -> p k f"))

        # ---- persistent working buffers ----
        # aA[g] serves as both block-k 'a' and block-k+1 'h' (normalize is
        # in place); Tile's WAR tracking orders the overwrites.
        aA = [hbufs.tile([128, L], F16, name=f"aA{g}") for g in range(8)]
        hW = [hbufs.tile([128, 16, 128], F16, name=f"hW{g}") for g in range(8)]
        hWx = hbufs.tile([128, 16, 128], F16, name="hWx")
        amx_glob = [hbufs.tile([128, L], F16, name=f"amxg{k}") for k in range(3)]
        amx_loc = hbufs.tile([128, L], F16, name="amx_loc")
        stats_sb = hbufs.tile([128, 192], F32, name="stats_sb")
        astat_sb = hbufs.tile([128, 24], F32, name="astat_sb")
        shared_agg = hbufs.tile([128, L], F32, name="shared_agg")

        def evac1(ps, g, ch, last_k=None):
            """block-1 evac: psum [128c, 512] -> fp16 a (ACT) + bn_stats
            (DVE, from psum) + running amax (DVE, from fp16 a). last_k:
            inject the prepay aggregation between bn_stats and max."""
            sl = slice(ch * 512, (ch + 1) * 512)
            nc.scalar.activation(aA[g][:, sl], ps, AF.Copy)
            nc.vector.bn_stats(stats_sb[:, (g * 4 + ch) * 6:(g * 4 + ch + 1) * 6],
                               ps)
            if last_k is not None:
                prepay(last_k)
            if g == 0:
                nc.vector.tensor_copy(amx_loc[:, sl], aA[g][:, sl])
            else:
                nc.vector.tensor_max(amx_loc[:, sl], amx_loc[:, sl],
                                     aA[g][:, sl])

        def evac2(ps, g, ch, last_k=None):
            """blocks-2/3 evac: aA = psum + shared_agg (DVE) + bn_stats on
            the fp16 result + running amax. The add is the only psum
            reader, so banks release fast. last_k: inject the prepay
            aggregation between bn_stats and max."""
            sl = slice(ch * 512, (ch + 1) * 512)
            nc.vector.tensor_add(aA[g][:, sl], ps, shared_agg[:, sl])
            nc.vector.bn_stats(stats_sb[:, (g * 4 + ch) * 6:(g * 4 + ch + 1) * 6],
                               aA[g][:, sl])
            if last_k is not None:
                prepay(last_k)
            if g == 0:
                nc.vector.tensor_copy(amx_loc[:, sl], aA[g][:, sl])
            else:
                nc.vector.tensor_max(amx_loc[:, sl], amx_loc[:, sl],
                                     aA[g][:, sl])

        def fire_amax_chunk(k, ch, in_eng=None, out_eng=None):
            """Pair AllReduce(max) of one 512-wide amax chunk. NB: no
            bn_stats here — it would head-of-line-block the strict-FIFO
            DVE queue. in_eng moves the bounce-in off the sync ring when
            outputs own it (tail). out_eng=scalar at boundaries puts the
            bounce-out BEHIND st1's payload DMA on the same ring so the
            ch3 pair-AR can never win the collective-channel race against
            the st1 AR (the race costs st1 ~13us when lost)."""
            sl = slice(ch * 512, (ch + 1) * 512)
            (out_eng or nc.sync).dma_start(amax_in_d[k][ch].ap(),
                                           amx_loc[:, sl])
            nc.gpsimd.collective_compute(
                "AllReduce", ALU.max, replica_groups=PAIRS,
                ins=[amax_in_d[k][ch].ap().opt()],
                outs=[amax_out_d[k][ch].ap().opt()])
            (in_eng or nc.sync).dma_start(amx_glob[k][:, sl],
                                          amax_out_d[k][ch].ap())

        pay1_t = {}

        def prepay(k):
            """Aggregate the aA-half stats payload. Called INSIDE the last
            evac of block k, between its bn_stats and its amax max: pay1
            is then ready BEFORE amx_loc's last chunk, so the scheduler's
            readiness-simulated ordering provably puts the st1 AR trigger
            ahead of the ch3 pair-AR on the collective channel."""
            loc1 = work.tile([128, 2], F32, name=f"loc1_{k}", tag="loc1")
            nc.vector.bn_aggr(loc1, stats_sb)
            pay1 = work.tile([128, 2], F32, name=f"pay1_{k}", tag="pay1")
            nc.vector.tensor_copy(pay1[:, 0:1], loc1[:, 0:1])
            nc.vector.scalar_tensor_tensor(pay1[:, 1:2], loc1[:, 0:1],
                                           loc1[:, 0:1], loc1[:, 1:2],
                                           ALU.mult, ALU.add)
            pay1_t[k] = pay1

        def affine1w(k, hi, gmean, gmsq, uid):
            """Folded BN+bias+relu per-channel (scale, shift) straight from
            the RAW AllReduce sums (no /8 op): 64*(var+eps) = 8*gmsq -
            gmean^2 + 64eps; sc = (8g)*rsqrt(64(var+eps)) with 8g folded
            on host; sh = be - (gmean/8 + bias)*sc."""
            g8_c = par_sb[:, k, hi:hi + 1]
            be_c = par_sb[:, k, 2 + hi:3 + hi]
            bias_c = par_sb[:, k, 4:5]
            t2 = work.tile([128, 1], F32, name=f"t2{uid}", tag="t2")
            nc.vector.tensor_mul(t2, gmean, gmean)
            v = work.tile([128, 1], F32, name=f"v{uid}", tag="v")
            nc.vector.scalar_tensor_tensor(v, gmsq, 8.0, t2,
                                           ALU.mult, ALU.subtract)
            nc.vector.tensor_scalar_add(v, v, 64.0 * EPS)
            r = work.tile([128, 1], F32, name=f"r{uid}", tag="r")
            nc.vector.reciprocal(r, v)
            me = work.tile([128, 1], F32, name=f"me{uid}", tag="me")
            nc.vector.scalar_tensor_tensor(me, gmean, 0.125, bias_c,
                                           ALU.mult, ALU.add)
            s = work.tile([128, 1], F32, name=f"s{uid}", tag="s")
            nc.scalar.sqrt(s, r)
            sc = work.tile([128, 1], F32, name=f"sc{uid}", tag="sc")
            nc.vector.tensor_mul(sc, s, g8_c)
            t4 = work.tile([128, 1], F32, name=f"t4{uid}", tag="t4")
            nc.vector.tensor_mul(t4, me, sc)
            sh = work.tile([128, 1], F32, name=f"sh{uid}", tag="sh")
            nc.vector.scalar_tensor_tensor(sh, t4, -1.0, be_c,
                                           ALU.mult, ALU.add)
            return sc, sh

        def boundary_head(k, in_eng=None):
            """End of block k: st1 AR the moment the last evac lands, then
            the ch3 pair AR, astats for the already-ARed chunks, and the
            st1-consume affine."""
            if k not in pay1_t:
                prepay(k)
            nc.scalar.dma_start(st1_in_d[k].ap(), pay1_t[k])
            nc.gpsimd.collective_compute(
                "AllReduce", ALU.add, replica_groups=ALLCORES,
                ins=[st1_in_d[k].ap().opt()], outs=[st1_out_d[k].ap().opt()])
            fire_amax_chunk(k, 3, in_eng=in_eng, out_eng=nc.scalar)
            for ch in range(3):
                nc.vector.bn_stats(astat_sb[:, ch * 6:(ch + 1) * 6],
                                   amx_glob[k][:, ch * 512:(ch + 1) * 512])
            gst1 = work.tile([128, 2], F32, name=f"gst1_{k}", tag="gst1")
            nc.scalar.dma_start(gst1, st1_out_d[k].ap())
            return affine1w(k, 0, gst1[:, 0:1], gst1[:, 1:2], f"a{k}0")

        def boundary_st2(k):
            """amax-half stats AR; emitted mid-psw so the DVE reaches the
            ch3 bn_stats right as its pair AR completes."""
            nc.vector.bn_stats(astat_sb[:, 18:24], amx_glob[k][:, 1536:2048])
            loc2 = work.tile([128, 2], F32, name=f"loc2_{k}", tag="loc2")
            nc.vector.bn_aggr(loc2, astat_sb)
            pay2 = work.tile([128, 2], F32, name=f"pay2_{k}", tag="pay2")
            nc.vector.tensor_copy(pay2[:, 0:1], loc2[:, 0:1])
            nc.vector.scalar_tensor_tensor(pay2[:, 1:2], loc2[:, 0:1],
                                           loc2[:, 0:1], loc2[:, 1:2],
                                           ALU.mult, ALU.add)
            nc.scalar.dma_start(st2_in_d[k].ap(), pay2)
            nc.gpsimd.collective_compute(
                "AllReduce", ALU.add, replica_groups=ALLCORES,
                ins=[st2_in_d[k].ap().opt()], outs=[st2_out_d[k].ap().opt()])

        def boundary_affine2(k):
            """st2 consume: amax-half affine + in-place relu of amx_glob[k],
            chunks split DVE/ACT. Emitted after the per-graph relus/psw
            evacs (no head-of-line blocking) and BEFORE any evac2 (which
            transitively waits on it via shared_agg — later would deadlock
            the DVE FIFO)."""
            gst2 = work.tile([128, 2], F32, name=f"gst2_{k}", tag="gst2")
            nc.scalar.dma_start(gst2, st2_out_d[k].ap())
            sc1, sh1 = affine1w(k, 1, gst2[:, 0:1], gst2[:, 1:2], f"a{k}1")
            am = amx_glob[k]
            for ch in (0, 1):
                sl = slice(ch * 512, (ch + 1) * 512)
                nc.vector.tensor_scalar(am[:, sl], am[:, sl], sc1, sh1,
                                        ALU.mult, ALU.add)
                nc.vector.tensor_scalar_max(am[:, sl], am[:, sl], 0.0)
            for ch in (2, 3):
                sl = slice(ch * 512, (ch + 1) * 512)
                nc.scalar.activation(am[:, sl], am[:, sl], AF.Relu,
                                     bias=sh1, scale=sc1)

        def relu_g(g, sc0, sh0):
            """in-place relu-affine of aA[g]; ACT for g<4, DVE for g>=4."""
            if g < 4:
                nc.scalar.activation(aA[g], aA[g], AF.Relu,
                                     bias=sh0, scale=sc0)
            else:
                nc.vector.tensor_scalar(aA[g], aA[g], sc0, sh0,
                                        ALU.mult, ALU.add)
                nc.vector.tensor_scalar_max(aA[g], aA[g], 0.0)

        def psw_g(g, W_sb):
            """per-graph W-matmul into hW[g] (no amax fold); evac engine
            matches the relu engine split."""
            for lt4 in range(4):
                ps = psum_w.tile([128, 4, 128], F32, name="ps_w", tag="ps_w")
                for q in range(4):
                    lt = lt4 * 4 + q
                    sl = slice(lt * 128, (lt + 1) * 128)
                    nc.tensor.matmul(ps[:, q, :], lhsT=aA[g][:, sl],
                                     rhs=W_sb[:, 0, :], start=True, stop=True)
                dst = hW[g][:, lt4 * 4:(lt4 + 1) * 4, :]
                if g < 4:
                    nc.scalar.activation(dst, ps, AF.Copy)
                else:
                    nc.vector.tensor_copy(dst, ps)

        def a_psum(g, ch):
            """16-MM contraction of hW[g] against A chunk ch (psw part)."""
            ps = psum_a.tile([128, 512], F32, name="ps_a", tag="ps_a")
            for st in range(16):
                nc.tensor.matmul(ps, lhsT=hW[g][:, st, :],
                                 rhs=A_sb[:, ch, st, :],
                                 start=(st == 0), stop=(st == 15))
            return ps

        def virtual_w(k):
            """hWx = relu_amax^T W[:,1] (16 tiles); ACT evacs."""
            for lt4 in range(4):
                psx = psum_w.tile([128, 4, 128], F32, name="ps_vw", tag="ps_w")
                for q in range(4):
                    lt = lt4 * 4 + q
                    sl = slice(lt * 128, (lt + 1) * 128)
                    nc.tensor.matmul(psx[:, q, :], lhsT=amx_glob[k][:, sl],
                                     rhs=W_sb_cur[0][:, 1, :],
                                     start=True, stop=True)
                nc.scalar.activation(hWx[:, lt4 * 4:(lt4 + 1) * 4, :], psx,
                                     AF.Copy)

        def virtual_a(ch):
            """shared_agg[:, ch] = sum_st hWx[st]^T A[st, ch], fp32."""
            psv = psum_w.tile([128, 512], F32, name="ps_va", tag="ps_w")
            for st in range(16):
                nc.tensor.matmul(psv, lhsT=hWx[:, st, :],
                                 rhs=A_sb[:, ch, st, :],
                                 start=(st == 0), stop=(st == 15))
            nc.scalar.activation(shared_agg[:, ch * 512:(ch + 1) * 512], psv,
                                 AF.Copy)

        W_sb_cur = [None]

        def block_body(bk, k_prev, W_sb, virtual=True):
            """blocks 2/3 (bk = 1, 2): boundary k_prev's combined-stats
            consume, per-graph relu+psw interleaved across ACT/DVE, then
            the A-phase with the virtual amax pass slotted per chunk."""
            W_sb_cur[0] = W_sb
            sc0, sh0 = boundary_head(k_prev)
            for ga, gv in ((0, 4), (1, 5)):
                relu_g(ga, sc0, sh0)
                relu_g(gv, sc0, sh0)
                psw_g(ga, W_sb)
                psw_g(gv, W_sb)
            boundary_st2(k_prev)
            for ga, gv in ((2, 6), (3, 7)):
                relu_g(ga, sc0, sh0)
                relu_g(gv, sc0, sh0)
                psw_g(ga, W_sb)
                psw_g(gv, W_sb)
            boundary_affine2(k_prev)
            if virtual:
                # A-phase, chunk-major. ch0: hold 6 psums while the amax
                # chain resolves; virtual pass lands shared_agg[ch0].
                held = [(g, a_psum(g, 0)) for g in (0, 4, 1, 5, 2, 6)]
                virtual_w(k_prev)
                virtual_a(0)
                for g, ps in held:
                    evac2(ps, g, 0)
                for g in (3, 7):
                    evac2(a_psum(g, 0), g, 0)
                fire_amax_chunk(bk, 0)
                for ch in range(1, 4):
                    virtual_a(ch)
                    for g in (0, 4, 1, 5, 2, 6, 3, 7):
                        evac2(a_psum(g, ch), g, ch)
                    if ch < 3:
                        fire_amax_chunk(bk, ch)
            else:
                # block 3: boundary-2's amax chain resolves early (no
                # backlog), so skip the virtual pass (+14us PE) and fold
                # hWx into hW with in-place DVE adds instead.
                virtual_w(k_prev)
                for g in (0, 4, 1, 5, 2, 6, 3, 7):
                    for lt4 in range(4):
                        sl4 = slice(lt4 * 4, (lt4 + 1) * 4)
                        nc.vector.tensor_add(hW[g][:, sl4, :],
                                             hW[g][:, sl4, :], hWx[:, sl4, :])
                for ch in range(4):
                    for g in (0, 4, 1, 5, 2, 6, 3, 7):
                        evac1(a_psum(g, ch), g, ch)
                    if ch < 3:
                        fire_amax_chunk(bk, ch)

        # ================= block 1 =================
        Ah_sb = blk1.tile([128, 4, 2048], F16)
        for ch in range(4):
            csl = slice(ch * 512, (ch + 1) * 512)
            for pk in range(4):
                ps = psum_a.tile([128, 512], F32, name="ps_a1", tag="ps_a")
                for st in range(16):
                    nc.tensor.matmul(ps, lhsT=x_sb[:, pk, st, :],
                                     rhs=A_sb[:, ch, st, :],
                                     start=(st == 0), stop=(st == 15))
                nc.scalar.activation(Ah_sb[:, pk, csl], ps, AF.Copy)
            for pk in range(4):
                for j in range(2):
                    g = 2 * pk + j
                    ps2 = psum_w.tile([128, 512], F32, name="ps_w1",
                                      tag="ps_w")
                    nc.tensor.matmul(ps2, lhsT=W1_sb[:, j, :],
                                     rhs=Ah_sb[:, pk, csl],
                                     start=True, stop=True)
                    evac1(ps2, g, ch)
            if ch < 3:
                fire_amax_chunk(0, ch)

        # ================= blocks 2 & 3 =================
        block_body(1, 0, W2_sb)
        block_body(2, 1, W3_sb, virtual=False)

        # ================= block-3 tail =================
        # st1 first: bulk relu + output DMA overlap the ch3/st2 chain.
        sc0, sh0 = boundary_head(2, in_eng=nc.scalar)

        def out_g(g):
            relu_g(g, sc0, sh0)
            nc.sync.dma_start(out_dram[g, :, 0:1024], aA[g][:, 0:1024])
            nc.sync.dma_start(out_dram[g, :, 1024:2048], aA[g][:, 1024:2048])

        for ga, gv in ((0, 4), (1, 5)):
            out_g(ga)
            out_g(gv)
        boundary_st2(2)
        for ga, gv in ((2, 6), (3, 7)):
            out_g(ga)
            out_g(gv)
        boundary_affine2(2)
        nc.scalar.dma_start(out2_dram[:, 0:1024], amx_glob[2][:, 0:1024])
        nc.scalar.dma_start(out2_dram[:, 1024:2048], amx_glob[2][:, 1024:2048])

    nc.compile()
    return nc


def _host_prep(x, edge_index, W1, b1, W2, b2, W3, b3,
               g1, be1, g2, be2, g3, be3):
    A_T = _build_A_T(edge_index).astype(np.float16)
    # [ch, p, st, j] = A_T[st*128+p, ch*512+j]
    a_t = np.ascontiguousarray(
        A_T.reshape(16, 128, 4, 512).transpose(2, 1, 0, 3))

    w1p = np.zeros([128, 2, 128], np.float16)
    w1p[0:64, 0, :] = W1
    w1p[64:128, 1, :] = W1
    w2c = np.ascontiguousarray(
        W2.astype(np.float16).reshape(2, 128, 128).transpose(1, 0, 2))
    w3c = np.ascontiguousarray(
        W3.astype(np.float16).reshape(2, 128, 128).transpose(1, 0, 2))

    # par columns: [8*g_h0, 8*g_h1, be_h0, be_h1, bias, bias] — pairs are
    # contiguous for the 2-wide affine; the 8x folds the /NCORES of the
    # stats AllReduce into the affine scale.
    par = np.zeros([3, 128, 6], np.float32)
    for k, (b_, g_, be_) in enumerate(
            [(b1, g1, be1), (b2, g2, be2), (b3, g3, be3)]):
        par[k, :, 0] = 8.0 * g_[:128]
        par[k, :, 1] = 8.0 * g_[128:]
        par[k, :, 2] = be_[:128]
        par[k, :, 3] = be_[128:]
        par[k, :, 4] = b_
        par[k, :, 5] = b_

    in_maps = []
    for core in range(NCORES):
        b_idx, nh = core // 2, core % 2
        xnm = np.ascontiguousarray(
            x[b_idx, nh * 8:nh * 8 + 8].transpose(0, 2, 1)).astype(np.float16)
        t = xnm.reshape(8, 16, 128, 64)  # [g, st, p, c]
        xp = np.zeros([128, 4, 16, 128], np.float16)
        for pk in range(4):
            xp[:, pk, :, 0:64] = t[2 * pk].transpose(1, 0, 2)
            xp[:, pk, :, 64:128] = t[2 * pk + 1].transpose(1, 0, 2)
        in_maps.append(dict(x_pack=xp, a_t=a_t, w1p=w1p, w2c=w2c, w3c=w3c,
                            par=par))
    return in_maps


def _get_nc():
    if "nc" not in _CACHE:
        _CACHE["nc"] = _build_nc()
    return _CACHE["nc"]


def _install_profiling_shim():
    """This image's antenv lacks axon_hooks; recreate it so trace=True works."""
    import sys
    import types
    if "antenv.axon_hooks" in sys.modules:
        return
    mod = types.ModuleType("antenv.axon_hooks")
    state = {"hook": None}
    mod.set_axon_ntff_profile_hook = lambda h: state.__setitem__("hook", h)
    mod.get_axon_ntff_profile_hook = lambda: state["hook"]
    sys.modules["antenv.axon_hooks"] = mod
    try:
        from trn_agent_boot.trn_boot import _ntff_profile_via_ctypes
        mod.set_axon_ntff_profile_hook(
            _ntff_profile_via_ctypes("/opt/axon/libaxon_pjrt.so"))
    except Exception:
        pass
    # zero-egress container: skip the artifact bucket upload
    import concourse.bass_utils as bu
    bu.upload_artifacts = lambda tmpdir: tmpdir


def _run(in_maps, trace=False):
    nc = _get_nc()
    kwargs = {}
    if trace:
        _install_profiling_shim()
        os.environ["BASS_PERFETTO_PROFILE_ALL_CORES"] = "1"
        kwargs["trace"] = True
    from concourse.bass_utils import run_bass_kernel_spmd
    res = run_bass_kernel_spmd(nc, in_maps, core_ids=list(range(NCORES)),
                               **kwargs)
    return res


def _assemble(results):
    out = np.zeros((B, N, 256, L), np.float32)
    for core in range(NCORES):
        b_idx, nh = core // 2, core % 2
        sl = slice(nh * 8, nh * 8 + 8)
        out[b_idx, sl, 0:128] = np.asarray(
            results[core]["out"]).reshape(8, 128, L).astype(np.float32)
        out[b_idx, sl, 128:256] = np.asarray(
            results[core]["out2"]).reshape(128, L).astype(np.float32)[None]
    return out.reshape(B * N, 256, L)


def kernel(**inputs):
    np_inputs = {k: np.asarray(v) for k, v in inputs.items()}
    in_maps = _host_prep(**{k: (np_inputs[k].astype(np.float32)
                                if k != "edge_index" else np_inputs[k])
                            for k in np_inputs})
    # The collective runtime very occasionally produces a corrupted run
    # (NaN/Inf in the output); re-dispatching the same compiled NEFF is
    # cheap and reliable, so guard with a couple of retries.
    for _ in range(3):
        res = _run(in_maps, trace=False)
        out = _assemble(res.results)
        if np.isfinite(out).all():
            break
    return out


def kernel_traced(**inputs):
    """Returns (output, BassKernelResults) with NTFF profiling if available."""
    np_inputs = {k: np.asarray(v) for k, v in inputs.items()}
    in_maps = _host_prep(**{k: (np_inputs[k].astype(np.float32)
                                if k != "edge_index" else np_inputs[k])
                            for k in np_inputs})
    res = _run(in_maps, trace=True)
    return _assemble(res.results), res
